# revision 16
# baseline (speedup 1.0000x reference)
"""ComplexPolarAttention Trainium2 kernel (8-core SPMD, row-sharded).

Math (matching the reference):
  c = mag*cos(phase); s = mag*sin(phase)
  scores = c@c.T + s@s.T + bias     (bias: sparse edge scatter, last-dup-wins)
  attn = softmax(scores, axis=1)
  out = (attn@mag, attn@phase)

Host prep: X^T = [c|s]^T packed as xt [128 feat, 8192 nodes] f16, the
per-core query slice xtq [128, 1024], and V = [mag|phase|ones] in key-chunk
layout mp [128, 64*132] bf16.  Edges are deduped (last wins) and bucketed
by (core, query-block, key-group, dst%128) for gpsimd local_scatter.

Device plan per core (1024 query rows), per (qb, g) group tile:
  S^T chunk [128 dst, 1024] = 4 matmuls  (f16, PSUM f32)
  P0 = exp(S^T) via ACT directly PSUM->SBUF bf16
  B  = local_scatter(expm1(edge_score))  (dense bf16 tile, zero background)
  P  = (B + 1) * P0   -- one DVE scalar_tensor_tensor in bf16 (4x mode)
  PV: out[q, 0:129] += P^T chunk @ [mag|phase|ones]  (bf16 matmuls)
  -> col 128 is the softmax denominator; epilogue multiplies by reciprocal.
PV matmuls lag the QK stream by PV_LAG groups so the PE never stalls on
the ACT/DVE exp chain.
"""
import os
import sys

sys.path.insert(0, "/opt/trn_rl_repo")

# The NTFF profile hook module is missing from this image's antenv package;
# bass_utils imports it unconditionally when tracing. Create it if absent so
# BASS_TRACE=1 works (degrades silently if dirs are read-only).
_HOOK_SRC = '''_hook = None

def set_axon_ntff_profile_hook(hook):
    global _hook
    _hook = hook

def get_axon_ntff_profile_hook():
    return _hook
'''
for _d in ("/opt/trn_rl_repo/antenv", "/root/.axon_site/_ro/trn_rl_repo/antenv"):
    try:
        _p = os.path.join(_d, "axon_hooks.py")
        if os.path.isdir(_d) and not os.path.exists(_p):
            with open(_p, "w") as _f:
                _f.write(_HOOK_SRC)
    except OSError:
        pass

import numpy as np
import ml_dtypes

import concourse.bass as bass
import concourse.mybir as mybir
import concourse.tile as tile
from concourse import bacc
from concourse import dve_ops as _dops
from concourse.bass_utils import run_bass_kernel_spmd
from concourse.dve_spec import Spec, Src0, Src1, Zero, eq, lower as _dve_lower
from concourse.dve_ops import has_src1 as _has_src1
from concourse.dve_uop import DveOpSpec as _DveOpSpec


def _register_edge_mult():
    """out = (in0 + (in0 == 0)) * in1 — multiplier with background 1.

    in0 is the sparse-scattered exp(edge_score) tile (zero background);
    in1 is exp(scores). One DVE pass instead of an stt + tensor_tensor."""
    name = "EDGE_MULT_ANT"
    for op in _dops.OPS:
        if op.name == name:
            return op
    spec = Spec(
        body=(Src0 + eq(Src0, Zero)) * Src1,
        reference=lambda in0, in1, s0, s1, imm2: (
            (in0 == 0).astype(np.float32) + in0.astype(np.float32)
        ) * in1.astype(np.float32),
    )
    opcode = _dops._CUSTOM_DVE_ROW_BASE + len(_dops.OPS)
    assert opcode < 0x20
    shas = {}
    for ver in ("v3", "v4"):
        tmp = _DveOpSpec(name=name, opcode=opcode,
                         uops=_dve_lower(spec, ver=ver),
                         rd1_en=_has_src1(spec))
        shas[ver] = tmp.sha(ver)
    op = _dops.DveOp(name, spec, subdim=False, uops_sha=shas)
    _dops.OPS.append(op)
    _dops._SUB_OPCODE_FOR_NAME[name] = opcode
    _dops.CUSTOM_DVE_SPECS[name] = spec
    return op


EDGE_MULT = _register_edge_mult()

N, D, E, EDGE_DIM = 8192, 64, 262144, 4
CORES = 8
NQ = N // CORES          # 1024 query rows per core
QB_W = 256               # query block width
N_QB = NQ // QB_W        # 4 query blocks per core
KC = 128                 # key chunk (dst) width
N_KC = N // KC           # 64 key chunks
KCG = 4                  # key chunks per scatter/exp group
N_G = N_KC // KCG        # 16 groups
GW = KCG * QB_W          # 1024 = group tile width
MPW = 132                # padded [mag|phase|ones] chunk stride
PV_LAG = 2               # PV matmuls trail QK by this many groups

f32 = mybir.dt.float32
f16 = mybir.dt.float16
bf16 = mybir.dt.bfloat16
i16 = mybir.dt.int16
AF = mybir.ActivationFunctionType
ALU = mybir.AluOpType

_cache = {}
LAST_RESULTS = None


def _build(slots):
    tot = N_QB * N_G * slots
    nc = bacc.Bacc("TRN2", target_bir_lowering=False, debug=False,
                   num_devices=CORES)
    xt_d = nc.dram_tensor("xt", (128, N), f16, kind="ExternalInput")
    xtq_d = nc.dram_tensor("xtq", (128, NQ), f16, kind="ExternalInput")
    mp_d = nc.dram_tensor("mp", (128, N_KC * MPW), bf16, kind="ExternalInput")
    eidx_d = nc.dram_tensor("eidx", (128, tot), i16, kind="ExternalInput")
    eattr_d = nc.dram_tensor("eattr", (128, 4 * tot), f16, kind="ExternalInput")
    w_d = nc.dram_tensor("W", (D, EDGE_DIM), f32, kind="ExternalInput")
    b_d = nc.dram_tensor("bvec", (D, 1), f32, kind="ExternalInput")
    out_d = nc.dram_tensor("out", (NQ, 128), f32, kind="ExternalOutput")

    with tile.TileContext(nc) as tc, \
         tc.tile_pool(name="persist", bufs=1) as pers:
        xt_sb = pers.tile([128, N], f16, tag="xt")
        xtq_sb = pers.tile([128, NQ], f16, tag="xtq")
        mp_sb = pers.tile([128, N_KC * MPW], bf16, tag="mp")
        esb = pers.tile([128, tot], bf16, tag="esb")
        eidx_sb = pers.tile([128, tot], i16, tag="eidx_sb")

        # ---- edge scores: es = edge_attr @ W.sum(0) + b.sum(); esb = exp(es)
        # W/b/edge DMAs go first: the eprep matmuls head the PE queue and the
        # first scatter needs esb, so these gate the whole pipeline start.
        with tc.tile_pool(name="eprep", bufs=1) as ep, \
             tc.tile_pool(name="eprep_ps", bufs=1, space="PSUM") as epp:
            # Prologue DMA order = queue round-robin; the critical chains are
            # (eidx, ea -> es chain -> first scatter) and (w/b, xtq, xt[0]
            # -> first QK).  Slice them so they land on parallel queues.
            QT = tot // 4
            for qt in range(4):
                sl = slice(qt * QT, (qt + 1) * QT)
                nc.sync.dma_start(out=eidx_sb[:, sl], in_=eidx_d[:, sl])
            w_sb = ep.tile([D, EDGE_DIM], f32, tag="w_sb")
            nc.sync.dma_start(out=w_sb[:], in_=w_d[:])
            b_sb = ep.tile([D, 1], f32, tag="b_sb")
            nc.sync.dma_start(out=b_sb[:], in_=b_d[:])
            ea = ep.tile([128, 4 * tot], f16, tag="ea")
            for qt in range(4):  # quarter-wise so the first scatters
                for ch in range(4):  # aren't gated on the full chain
                    sl = slice(ch * tot + qt * QT, ch * tot + (qt + 1) * QT)
                    nc.sync.dma_start(out=ea[:, sl], in_=eattr_d[:, sl])
            for h in range(4):
                c0, c1 = h * (NQ // 4), (h + 1) * (NQ // 4)
                nc.sync.dma_start(out=xtq_sb[:, c0:c1], in_=xtq_d[:, c0:c1])
            for h in range(16):
                c0, c1 = h * (N // 16), (h + 1) * (N // 16)
                nc.sync.dma_start(out=xt_sb[:, c0:c1], in_=xt_d[:, c0:c1])
            for h in range(8):
                c0 = h * (N_KC // 8) * MPW
                c1 = (h + 1) * (N_KC // 8) * MPW
                nc.sync.dma_start(out=mp_sb[:, c0:c1], in_=mp_d[:, c0:c1])
            ones64 = ep.tile([D, 1], f32, tag="ones64")
            nc.vector.memset(ones64[:], 1.0)
            ones1 = ep.tile([1, 128], f32, tag="ones1")
            nc.vector.memset(ones1[:], 1.0)

            ws_ps = epp.tile([1, EDGE_DIM], f32, tag="ws_ps")
            nc.tensor.matmul(out=ws_ps[:], lhsT=ones64[:], rhs=w_sb[:],
                             start=True, stop=True)
            ws_row = ep.tile([1, EDGE_DIM], f32, tag="ws_row")
            nc.scalar.copy(out=ws_row[:], in_=ws_ps[:])
            bs_ps = epp.tile([1, 1], f32, tag="bs_ps")
            nc.tensor.matmul(out=bs_ps[:], lhsT=b_sb[:], rhs=ones64[:],
                             start=True, stop=True)
            bs_row = ep.tile([1, 1], f32, tag="bs_row")
            nc.scalar.copy(out=bs_row[:], in_=bs_ps[:])
            wbc_ps = epp.tile([128, EDGE_DIM], f32, tag="wbc_ps")
            nc.tensor.matmul(out=wbc_ps[:], lhsT=ones1[:], rhs=ws_row[:],
                             start=True, stop=True)
            wbc = ep.tile([128, EDGE_DIM], f32, tag="wbc")
            nc.scalar.copy(out=wbc[:], in_=wbc_ps[:])
            bbc_ps = epp.tile([128, 1], f32, tag="bbc_ps")
            nc.tensor.matmul(out=bbc_ps[:], lhsT=ones1[:], rhs=bs_row[:],
                             start=True, stop=True)
            bbc = ep.tile([128, 1], f32, tag="bbc")
            nc.scalar.copy(out=bbc[:], in_=bbc_ps[:])

            acc_a = ep.tile([128, tot], f32, tag="acc_a")
            acc_b = ep.tile([128, tot], f32, tag="acc_b")
            for qt in range(4):
                q0, q1 = qt * QT, (qt + 1) * QT
                nc.vector.tensor_scalar(acc_a[:, q0:q1], ea[:, q0:q1],
                                        wbc[:, 0:1], None, ALU.mult)
                nc.vector.scalar_tensor_tensor(
                    out=acc_b[:, q0:q1], in0=ea[:, tot + q0:tot + q1],
                    scalar=wbc[:, 1:2], in1=acc_a[:, q0:q1],
                    op0=ALU.mult, op1=ALU.add)
                nc.vector.scalar_tensor_tensor(
                    out=acc_a[:, q0:q1], in0=ea[:, 2 * tot + q0:2 * tot + q1],
                    scalar=wbc[:, 2:3], in1=acc_b[:, q0:q1],
                    op0=ALU.mult, op1=ALU.add)
                nc.vector.scalar_tensor_tensor(
                    out=acc_b[:, q0:q1], in0=ea[:, 3 * tot + q0:3 * tot + q1],
                    scalar=wbc[:, 3:4], in1=acc_a[:, q0:q1],
                    op0=ALU.mult, op1=ALU.add)
                # es = acc_b + bbc; esb = exp(es)  (bf16, always > 0)
                nc.scalar.activation(out=esb[:, q0:q1], in_=acc_b[:, q0:q1],
                                     func=AF.Exp, bias=bbc[:, 0:1])

        # ---- main loop: QK -> exp -> scatter-mult -> PV (software pipelined)
        with tc.tile_pool(name="b_psb", bufs=2) as psbp, \
             tc.tile_pool(name="b_bias", bufs=8) as biasp, \
             tc.tile_pool(name="b_qk", bufs=3, space="PSUM") as qkpp, \
             tc.tile_pool(name="b_pv", bufs=1, space="PSUM") as pvpp, \
             tc.tile_pool(name="b_ep", bufs=2) as ep2:
            for qb in range(N_QB):
                p_sb = psbp.tile([128, N_KC * QB_W], bf16, tag="p_sb")
                pv0 = pvpp.tile([128, 129], f32, tag="pv0")
                pv1 = pvpp.tile([128, 129], f32, tag="pv1")

                def do_qk(g, qb=qb, p_sb=p_sb):
                    qk = qkpp.tile([128, GW], f32, tag="qk")
                    for j in range(KCG):
                        kc = g * KCG + j
                        nc.tensor.matmul(out=qk[:, j * QB_W:(j + 1) * QB_W],
                                         lhsT=xt_sb[:, kc * 128:(kc + 1) * 128],
                                         rhs=xtq_sb[:, qb * QB_W:(qb + 1) * QB_W],
                                         start=True, stop=True)
                    pslice = p_sb[:, g * GW:(g + 1) * GW]
                    nc.scalar.activation(out=pslice, in_=qk[:], func=AF.Exp)
                    bias_t = biasp.tile([128, GW], bf16, tag="bias_t")
                    off = (qb * N_G + g) * slots
                    nc.gpsimd.local_scatter(bias_t[:], esb[:, off:off + slots],
                                            eidx_sb[:, off:off + slots],
                                            channels=128, num_elems=GW,
                                            num_idxs=slots)
                    # p *= (bias + (bias==0)): multiplier exp(es) at edge
                    # cells, 1 elsewhere (a 1+expm1 encoding in bf16 suffers
                    # catastrophic cancellation for negative edge scores)
                    nc.vector._custom_dve(
                        EDGE_MULT, out=pslice, in0=bias_t[:], in1=pslice)

                def do_pv(g, p_sb=p_sb, pv0=pv0, pv1=pv1):
                    for j in range(KCG):
                        kc = g * KCG + j
                        for qs, pv in ((0, pv0), (1, pv1)):
                            nc.tensor.matmul(
                                out=pv[:],
                                lhsT=p_sb[:, kc * QB_W + qs * 128:
                                          kc * QB_W + (qs + 1) * 128],
                                rhs=mp_sb[:, kc * MPW:kc * MPW + 2 * D + 1],
                                start=(kc == 0), stop=(kc == N_KC - 1))

                for g in range(N_G):
                    do_qk(g)
                    if g >= PV_LAG:
                        do_pv(g - PV_LAG)
                for g in range(N_G - PV_LAG, N_G):
                    do_pv(g)

                for qs, pv in ((0, pv0), (1, pv1)):
                    rec = ep2.tile([128, 1], f32, tag=f"rec{qs}")
                    nc.vector.reciprocal(out=rec[:], in_=pv[:, 128:129])
                    o_t = ep2.tile([128, 128], f32, tag=f"o_t{qs}")
                    nc.scalar.mul(out=o_t[:], in_=pv[:, 0:128], mul=rec[:])
                    r0 = qb * QB_W + qs * 128
                    nc.sync.dma_start(out=out_d[r0:r0 + 128, :], in_=o_t[:])

    nc.compile()
    return nc


def _prep_edges(edge_index, edge_attr):
    """Dedup (last wins, matching CPU XLA scatter-set) and bucket edges."""
    src = np.asarray(edge_index[0], dtype=np.int64)
    dst = np.asarray(edge_index[1], dtype=np.int64)
    keys = src * N + dst
    order = np.argsort(keys, kind="stable")
    ks = keys[order]
    run_last = np.flatnonzero(np.r_[ks[1:] != ks[:-1], True])
    kept = order[run_last]  # stable sort => last occurrence per duplicate key
    s, d = src[kept], dst[kept]
    attr = np.asarray(edge_attr, dtype=np.float32)[kept]

    core = s // NQ
    qb = (s % NQ) // QB_W
    g = d // (KCG * KC)
    p = d % 128
    col = ((d % (KCG * KC)) // KC) * QB_W + (s % QB_W)

    cell = ((core * N_QB + qb) * N_G + g) * 128 + p
    o2 = np.argsort(cell, kind="stable")
    cell_s = cell[o2]
    first = np.r_[True, cell_s[1:] != cell_s[:-1]]
    run_id = np.cumsum(first) - 1
    run_start = np.flatnonzero(first)
    slot = np.arange(len(cell_s)) - run_start[run_id]
    slots = int(max(int(slot.max()) + 1 if len(slot) else 1, 4))
    slots = (slots + 1) // 2 * 2  # even

    tot = N_QB * N_G * slots
    eidx = np.full((CORES, 128, tot), -1, dtype=np.int16)
    eattr = np.zeros((CORES, 128, 4, tot), dtype=np.float16)
    cs, qbs, gs, ps, cols = core[o2], qb[o2], g[o2], p[o2], col[o2]
    off = (qbs * N_G + gs) * slots + slot
    eidx[cs, ps, off] = cols.astype(np.int16)
    a2 = attr[o2]
    for ch in range(EDGE_DIM):
        eattr[cs, ps, ch, off] = a2[:, ch].astype(np.float16)
    return eidx, eattr.reshape(CORES, 128, 4 * tot), slots


def kernel(mag, phase, edge_index, edge_attr, W, b):
    global LAST_RESULTS
    mag = np.ascontiguousarray(np.asarray(mag, dtype=np.float32))
    phase = np.ascontiguousarray(np.asarray(phase, dtype=np.float32))
    W = np.ascontiguousarray(np.asarray(W, dtype=np.float32))
    bvec = np.ascontiguousarray(np.asarray(b, dtype=np.float32).reshape(D, 1))

    # xt = [c|s]^T in f16, mp = [mag|phase|ones|pad] in key-chunk layout bf16
    c = mag * np.cos(phase)
    s = mag * np.sin(phase)
    xt = np.ascontiguousarray(
        np.concatenate([c, s], axis=1).T.astype(np.float16))  # [128, N]
    v = np.concatenate(
        [mag, phase, np.ones((N, 1), np.float32), np.zeros((N, 3), np.float32)],
        axis=1)  # [N, 132]
    mp = np.ascontiguousarray(
        v.reshape(N_KC, 128, MPW).transpose(1, 0, 2)
        .reshape(128, N_KC * MPW).astype(ml_dtypes.bfloat16))

    eidx, eattr, slots = _prep_edges(edge_index, edge_attr)

    if slots not in _cache:
        _cache[slots] = _build(slots)
    nc = _cache[slots]

    in_maps = []
    for cid in range(CORES):
        in_maps.append({
            "xt": xt,
            "xtq": np.ascontiguousarray(xt[:, cid * NQ:(cid + 1) * NQ]),
            "mp": mp,
            "eidx": np.ascontiguousarray(eidx[cid]),
            "eattr": np.ascontiguousarray(eattr[cid]),
            "W": W, "bvec": bvec,
        })
    res = run_bass_kernel_spmd(nc, in_maps, core_ids=list(range(CORES)))
    LAST_RESULTS = res

    new_mag = np.empty((N, D), dtype=np.float32)
    new_phase = np.empty((N, D), dtype=np.float32)
    for cid in range(CORES):
        o = res.results[cid]["out"]
        new_mag[cid * NQ:(cid + 1) * NQ] = o[:, 0:D]
        new_phase[cid * NQ:(cid + 1) * NQ] = o[:, D:2 * D]
    return new_mag, new_phase


# revision 19
# speedup vs baseline: 1.1038x; 1.1038x over previous
"""ComplexPolarAttention Trainium2 kernel (8-core SPMD, row-sharded).

Math (matching the reference):
  c = mag*cos(phase); s = mag*sin(phase)
  scores = c@c.T + s@s.T + bias     (bias: sparse edge scatter, last-dup-wins)
  attn = softmax(scores, axis=1)
  out = (attn@mag, attn@phase)

Host prep: X^T = [c|s]^T packed as xt [128 feat, 8192 nodes] f16, the
per-core query slice xtq [128, 1024], and V = [mag|phase|ones] in key-chunk
layout mp [128, 64*132] bf16.  Edges are deduped (last wins) and bucketed
by (core, query-block, key-group, dst%128) for gpsimd local_scatter.

Device plan per core (1024 query rows), per (qb, g) group tile:
  S^T chunk [128 dst, 1024] = 4 matmuls  (f16, PSUM f32)
  P0 = exp(S^T) via ACT directly PSUM->SBUF bf16
  B  = local_scatter(expm1(edge_score))  (dense bf16 tile, zero background)
  P  = (B + 1) * P0   -- one DVE scalar_tensor_tensor in bf16 (4x mode)
  PV: out[q, 0:129] += P^T chunk @ [mag|phase|ones]  (bf16 matmuls)
  -> col 128 is the softmax denominator; epilogue multiplies by reciprocal.
PV matmuls lag the QK stream by PV_LAG groups so the PE never stalls on
the ACT/DVE exp chain.
"""
import os
import sys

sys.path.insert(0, "/opt/trn_rl_repo")

# The NTFF profile hook module is missing from this image's antenv package;
# bass_utils imports it unconditionally when tracing. Create it if absent so
# BASS_TRACE=1 works (degrades silently if dirs are read-only).
_HOOK_SRC = '''_hook = None

def set_axon_ntff_profile_hook(hook):
    global _hook
    _hook = hook

def get_axon_ntff_profile_hook():
    return _hook
'''
for _d in ("/opt/trn_rl_repo/antenv", "/root/.axon_site/_ro/trn_rl_repo/antenv"):
    try:
        _p = os.path.join(_d, "axon_hooks.py")
        if os.path.isdir(_d) and not os.path.exists(_p):
            with open(_p, "w") as _f:
                _f.write(_HOOK_SRC)
    except OSError:
        pass

import numpy as np
import ml_dtypes

import concourse.bass as bass
import concourse.mybir as mybir
import concourse.tile as tile
from concourse import bacc
from concourse import dve_ops as _dops
from concourse.bass_utils import run_bass_kernel_spmd
from concourse.dve_spec import Spec, Src0, Src1, Zero, eq, lower as _dve_lower
from concourse.dve_ops import has_src1 as _has_src1
from concourse.dve_uop import DveOpSpec as _DveOpSpec


def _register_edge_mult():
    """out = (in0 + (in0 == 0)) * in1 — multiplier with background 1.

    in0 is the sparse-scattered exp(edge_score) tile (zero background);
    in1 is exp(scores). One DVE pass instead of an stt + tensor_tensor."""
    name = "EDGE_MULT_ANT"
    for op in _dops.OPS:
        if op.name == name:
            return op
    spec = Spec(
        body=(Src0 + eq(Src0, Zero)) * Src1,
        reference=lambda in0, in1, s0, s1, imm2: (
            (in0 == 0).astype(np.float32) + in0.astype(np.float32)
        ) * in1.astype(np.float32),
    )
    opcode = _dops._CUSTOM_DVE_ROW_BASE + len(_dops.OPS)
    assert opcode < 0x20
    shas = {}
    for ver in ("v3", "v4"):
        tmp = _DveOpSpec(name=name, opcode=opcode,
                         uops=_dve_lower(spec, ver=ver),
                         rd1_en=_has_src1(spec))
        shas[ver] = tmp.sha(ver)
    op = _dops.DveOp(name, spec, subdim=False, uops_sha=shas)
    _dops.OPS.append(op)
    _dops._SUB_OPCODE_FOR_NAME[name] = opcode
    _dops.CUSTOM_DVE_SPECS[name] = spec
    return op


EDGE_MULT = _register_edge_mult()

N, D, E, EDGE_DIM = 8192, 64, 262144, 4
CORES = 8
NQ = N // CORES          # 1024 query rows per core
QB_W = 256               # query block width
N_QB = NQ // QB_W        # 4 query blocks per core
KC = 128                 # key chunk (dst) width
N_KC = N // KC           # 64 key chunks
KCG = 4                  # key chunks per scatter/exp group
N_G = N_KC // KCG        # 16 groups
GW = KCG * QB_W          # 1024 = group tile width
MPW = 132                # padded [mag|phase|ones] chunk stride
PV_LAG = 2               # PV matmuls trail QK by this many groups

f32 = mybir.dt.float32
f16 = mybir.dt.float16
bf16 = mybir.dt.bfloat16
i16 = mybir.dt.int16
AF = mybir.ActivationFunctionType
ALU = mybir.AluOpType

_cache = {}
LAST_RESULTS = None


def _build(slots):
    tot = N_QB * N_G * slots
    nc = bacc.Bacc("TRN2", target_bir_lowering=False, debug=False,
                   num_devices=CORES)
    xt_d = nc.dram_tensor("xt", (128, N), f16, kind="ExternalInput")
    xtq_d = nc.dram_tensor("xtq", (128, NQ), f16, kind="ExternalInput")
    mp_d = nc.dram_tensor("mp", (128, N_KC * MPW), bf16, kind="ExternalInput")
    eidx_d = nc.dram_tensor("eidx", (128, tot), i16, kind="ExternalInput")
    eattr_d = nc.dram_tensor("eattr", (128, 4 * tot), f16, kind="ExternalInput")
    w_d = nc.dram_tensor("W", (D, EDGE_DIM), f32, kind="ExternalInput")
    b_d = nc.dram_tensor("bvec", (D, 1), f32, kind="ExternalInput")
    out_d = nc.dram_tensor("out", (NQ, 128), f32, kind="ExternalOutput")

    with tile.TileContext(nc) as tc, \
         tc.tile_pool(name="persist", bufs=1) as pers:
        xt_sb = pers.tile([128, N], f16, tag="xt")
        xtq_sb = pers.tile([128, NQ], f16, tag="xtq")
        mp_sb = pers.tile([128, N_KC * MPW], bf16, tag="mp")
        esb = pers.tile([128, tot], bf16, tag="esb")
        eidx_sb = pers.tile([128, tot], i16, tag="eidx_sb")

        # ---- edge scores: es = edge_attr @ W.sum(0) + b.sum(); esb = exp(es)
        # W/b/edge DMAs go first: the eprep matmuls head the PE queue and the
        # first scatter needs esb, so these gate the whole pipeline start.
        with tc.tile_pool(name="eprep", bufs=1) as ep, \
             tc.tile_pool(name="eprep_ps", bufs=1, space="PSUM") as epp:
            # Each dma_start costs ~565ns of SP sequencer issue time, so keep
            # the count low and issue critical-path tensors first.  eattr is
            # host-reordered quarter-major: one DMA covers the whole first
            # quarter of the es chain (which gates the first scatter).
            QT = tot // 4
            w_sb = ep.tile([D, EDGE_DIM], f32, tag="w_sb")
            nc.sync.dma_start(out=w_sb[:], in_=w_d[:])
            b_sb = ep.tile([D, 1], f32, tag="b_sb")
            nc.sync.dma_start(out=b_sb[:], in_=b_d[:])
            nc.sync.dma_start(out=eidx_sb[:], in_=eidx_d[:])
            ea = ep.tile([128, 4 * tot], f16, tag="ea")
            for qt in range(4):
                sl = slice(4 * qt * QT, 4 * (qt + 1) * QT)
                nc.sync.dma_start(out=ea[:, sl], in_=eattr_d[:, sl])
            nc.sync.dma_start(out=xtq_sb[:], in_=xtq_d[:])
            for h in range(4):
                c0, c1 = h * (N // 4), (h + 1) * (N // 4)
                nc.sync.dma_start(out=xt_sb[:, c0:c1], in_=xt_d[:, c0:c1])
            for h in range(2):
                c0 = h * (N_KC // 2) * MPW
                c1 = (h + 1) * (N_KC // 2) * MPW
                nc.sync.dma_start(out=mp_sb[:, c0:c1], in_=mp_d[:, c0:c1])
            ones64 = ep.tile([D, 1], f32, tag="ones64")
            nc.vector.memset(ones64[:], 1.0)
            ones1 = ep.tile([1, 128], f32, tag="ones1")
            nc.vector.memset(ones1[:], 1.0)

            ws_ps = epp.tile([1, EDGE_DIM], f32, tag="ws_ps")
            nc.tensor.matmul(out=ws_ps[:], lhsT=ones64[:], rhs=w_sb[:],
                             start=True, stop=True)
            ws_row = ep.tile([1, EDGE_DIM], f32, tag="ws_row")
            nc.scalar.copy(out=ws_row[:], in_=ws_ps[:])
            bs_ps = epp.tile([1, 1], f32, tag="bs_ps")
            nc.tensor.matmul(out=bs_ps[:], lhsT=b_sb[:], rhs=ones64[:],
                             start=True, stop=True)
            bs_row = ep.tile([1, 1], f32, tag="bs_row")
            nc.scalar.copy(out=bs_row[:], in_=bs_ps[:])
            wbc_ps = epp.tile([128, EDGE_DIM], f32, tag="wbc_ps")
            nc.tensor.matmul(out=wbc_ps[:], lhsT=ones1[:], rhs=ws_row[:],
                             start=True, stop=True)
            wbc = ep.tile([128, EDGE_DIM], f32, tag="wbc")
            nc.scalar.copy(out=wbc[:], in_=wbc_ps[:])
            bbc_ps = epp.tile([128, 1], f32, tag="bbc_ps")
            nc.tensor.matmul(out=bbc_ps[:], lhsT=ones1[:], rhs=bs_row[:],
                             start=True, stop=True)
            bbc = ep.tile([128, 1], f32, tag="bbc")
            nc.scalar.copy(out=bbc[:], in_=bbc_ps[:])

            acc_a = ep.tile([128, tot], f32, tag="acc_a")
            acc_b = ep.tile([128, tot], f32, tag="acc_b")
            for qt in range(4):
                q0, q1 = qt * QT, (qt + 1) * QT
                e0 = 4 * qt * QT  # quarter-major eattr: ch at e0 + ch*QT
                nc.vector.tensor_scalar(acc_a[:, q0:q1], ea[:, e0:e0 + QT],
                                        wbc[:, 0:1], None, ALU.mult)
                nc.vector.scalar_tensor_tensor(
                    out=acc_b[:, q0:q1], in0=ea[:, e0 + QT:e0 + 2 * QT],
                    scalar=wbc[:, 1:2], in1=acc_a[:, q0:q1],
                    op0=ALU.mult, op1=ALU.add)
                nc.vector.scalar_tensor_tensor(
                    out=acc_a[:, q0:q1], in0=ea[:, e0 + 2 * QT:e0 + 3 * QT],
                    scalar=wbc[:, 2:3], in1=acc_b[:, q0:q1],
                    op0=ALU.mult, op1=ALU.add)
                nc.vector.scalar_tensor_tensor(
                    out=acc_b[:, q0:q1], in0=ea[:, e0 + 3 * QT:e0 + 4 * QT],
                    scalar=wbc[:, 3:4], in1=acc_a[:, q0:q1],
                    op0=ALU.mult, op1=ALU.add)
                # es = acc_b + bbc; esb = exp(es)  (bf16, always > 0)
                nc.scalar.activation(out=esb[:, q0:q1], in_=acc_b[:, q0:q1],
                                     func=AF.Exp, bias=bbc[:, 0:1])

        # ---- main loop: QK -> exp -> scatter-mult -> PV (software pipelined)
        with tc.tile_pool(name="b_psb", bufs=2) as psbp, \
             tc.tile_pool(name="b_bias", bufs=8) as biasp, \
             tc.tile_pool(name="b_qk", bufs=3, space="PSUM") as qkpp, \
             tc.tile_pool(name="b_pv", bufs=1, space="PSUM") as pvpp, \
             tc.tile_pool(name="b_ep", bufs=2) as ep2:
            for qb in range(N_QB):
                p_sb = psbp.tile([128, N_KC * QB_W], bf16, tag="p_sb")
                pv0 = pvpp.tile([128, 129], f32, tag="pv0")
                pv1 = pvpp.tile([128, 129], f32, tag="pv1")

                def do_qk(g, qb=qb, p_sb=p_sb):
                    qk = qkpp.tile([128, GW], f32, tag="qk")
                    for j in range(KCG):
                        kc = g * KCG + j
                        nc.tensor.matmul(out=qk[:, j * QB_W:(j + 1) * QB_W],
                                         lhsT=xt_sb[:, kc * 128:(kc + 1) * 128],
                                         rhs=xtq_sb[:, qb * QB_W:(qb + 1) * QB_W],
                                         start=True, stop=True)
                    pslice = p_sb[:, g * GW:(g + 1) * GW]
                    nc.scalar.activation(out=pslice, in_=qk[:], func=AF.Exp)
                    bias_t = biasp.tile([128, GW], bf16, tag="bias_t")
                    off = (qb * N_G + g) * slots
                    nc.gpsimd.local_scatter(bias_t[:], esb[:, off:off + slots],
                                            eidx_sb[:, off:off + slots],
                                            channels=128, num_elems=GW,
                                            num_idxs=slots)
                    # p *= (bias + (bias==0)): multiplier exp(es) at edge
                    # cells, 1 elsewhere (a 1+expm1 encoding in bf16 suffers
                    # catastrophic cancellation for negative edge scores)
                    nc.vector._custom_dve(
                        EDGE_MULT, out=pslice, in0=bias_t[:], in1=pslice)

                def do_pv(g, p_sb=p_sb, pv0=pv0, pv1=pv1):
                    for j in range(KCG):
                        kc = g * KCG + j
                        for qs, pv in ((0, pv0), (1, pv1)):
                            nc.tensor.matmul(
                                out=pv[:],
                                lhsT=p_sb[:, kc * QB_W + qs * 128:
                                          kc * QB_W + (qs + 1) * 128],
                                rhs=mp_sb[:, kc * MPW:kc * MPW + 2 * D + 1],
                                start=(kc == 0), stop=(kc == N_KC - 1))

                for g in range(N_G):
                    do_qk(g)
                    if g >= PV_LAG:
                        do_pv(g - PV_LAG)
                for g in range(N_G - PV_LAG, N_G):
                    do_pv(g)

                for qs, pv in ((0, pv0), (1, pv1)):
                    rec = ep2.tile([128, 1], f32, tag=f"rec{qs}")
                    nc.vector.reciprocal(out=rec[:], in_=pv[:, 128:129])
                    o_t = ep2.tile([128, 128], f32, tag=f"o_t{qs}")
                    nc.scalar.mul(out=o_t[:], in_=pv[:, 0:128], mul=rec[:])
                    r0 = qb * QB_W + qs * 128
                    nc.sync.dma_start(out=out_d[r0:r0 + 128, :], in_=o_t[:])

    nc.compile()
    return nc


def _prep_edges(edge_index, edge_attr):
    """Dedup (last wins, matching CPU XLA scatter-set) and bucket edges."""
    src = np.asarray(edge_index[0], dtype=np.int64)
    dst = np.asarray(edge_index[1], dtype=np.int64)
    keys = src * N + dst
    order = np.argsort(keys, kind="stable")
    ks = keys[order]
    run_last = np.flatnonzero(np.r_[ks[1:] != ks[:-1], True])
    kept = order[run_last]  # stable sort => last occurrence per duplicate key
    s, d = src[kept], dst[kept]
    attr = np.asarray(edge_attr, dtype=np.float32)[kept]

    core = s // NQ
    qb = (s % NQ) // QB_W
    g = d // (KCG * KC)
    p = d % 128
    col = ((d % (KCG * KC)) // KC) * QB_W + (s % QB_W)

    cell = ((core * N_QB + qb) * N_G + g) * 128 + p
    o2 = np.argsort(cell, kind="stable")
    cell_s = cell[o2]
    first = np.r_[True, cell_s[1:] != cell_s[:-1]]
    run_id = np.cumsum(first) - 1
    run_start = np.flatnonzero(first)
    slot = np.arange(len(cell_s)) - run_start[run_id]
    slots = int(max(int(slot.max()) + 1 if len(slot) else 1, 4))
    slots = (slots + 1) // 2 * 2  # even

    tot = N_QB * N_G * slots
    eidx = np.full((CORES, 128, tot), -1, dtype=np.int16)
    eattr = np.zeros((CORES, 128, 4, tot), dtype=np.float16)
    cs, qbs, gs, ps, cols = core[o2], qb[o2], g[o2], p[o2], col[o2]
    off = (qbs * N_G + gs) * slots + slot
    eidx[cs, ps, off] = cols.astype(np.int16)
    a2 = attr[o2]
    for ch in range(EDGE_DIM):
        eattr[cs, ps, ch, off] = a2[:, ch].astype(np.float16)
    # quarter(qb)-major channel layout: [qb][ch][N_G*slots] so one DMA
    # covers everything the qb'th quarter of the device es chain reads
    eattr = (eattr.reshape(CORES, 128, 4, N_QB, N_G * slots)
             .transpose(0, 1, 3, 2, 4).reshape(CORES, 128, 4 * tot))
    return eidx, eattr, slots


def kernel(mag, phase, edge_index, edge_attr, W, b):
    global LAST_RESULTS
    mag = np.ascontiguousarray(np.asarray(mag, dtype=np.float32))
    phase = np.ascontiguousarray(np.asarray(phase, dtype=np.float32))
    W = np.ascontiguousarray(np.asarray(W, dtype=np.float32))
    bvec = np.ascontiguousarray(np.asarray(b, dtype=np.float32).reshape(D, 1))

    # xt = [c|s]^T in f16, mp = [mag|phase|ones|pad] in key-chunk layout bf16
    c = mag * np.cos(phase)
    s = mag * np.sin(phase)
    xt = np.ascontiguousarray(
        np.concatenate([c, s], axis=1).T.astype(np.float16))  # [128, N]
    v = np.concatenate(
        [mag, phase, np.ones((N, 1), np.float32), np.zeros((N, 3), np.float32)],
        axis=1)  # [N, 132]
    mp = np.ascontiguousarray(
        v.reshape(N_KC, 128, MPW).transpose(1, 0, 2)
        .reshape(128, N_KC * MPW).astype(ml_dtypes.bfloat16))

    eidx, eattr, slots = _prep_edges(edge_index, edge_attr)

    if slots not in _cache:
        _cache[slots] = _build(slots)
    nc = _cache[slots]

    in_maps = []
    for cid in range(CORES):
        in_maps.append({
            "xt": xt,
            "xtq": np.ascontiguousarray(xt[:, cid * NQ:(cid + 1) * NQ]),
            "mp": mp,
            "eidx": np.ascontiguousarray(eidx[cid]),
            "eattr": np.ascontiguousarray(eattr[cid]),
            "W": W, "bvec": bvec,
        })
    res = run_bass_kernel_spmd(nc, in_maps, core_ids=list(range(CORES)))
    LAST_RESULTS = res

    new_mag = np.empty((N, D), dtype=np.float32)
    new_phase = np.empty((N, D), dtype=np.float32)
    for cid in range(CORES):
        o = res.results[cid]["out"]
        new_mag[cid * NQ:(cid + 1) * NQ] = o[:, 0:D]
        new_phase[cid * NQ:(cid + 1) * NQ] = o[:, D:2 * D]
    return new_mag, new_phase
